# revision 23
# baseline (speedup 1.0000x reference)
"""DetNet Trainium2 kernel v10 (all-f16, 127/113 m-split to defeat FWL): 90-layer MLP recurrence, data-parallel over 8 cores.

Per core (2048 samples), features on partitions, batch on free axis.
Partition layouts keep every engine op same-base (non-0 bases 32-aligned,
<=32 rows):
  cA   [126,2048] f32r: v(0:60) | pad | tH(64:94) | pad | Hr(96:126)
  trep [120,2048] f16 : t replicated 4x (block j4 at partitions 30*j4+k);
       produced rep4 for free by the W2-rep4 matmul (M is free on PE)
  ttr  [120,2048] f32 : t_tilde state, rep4

Three-stage chunk pipeline per layer (keeps PE busy across the
sel->tH-copy->mm1 and mm1->relu->mm23 cross-engine dependencies):
  stage0(c):   DVE products, 8 selector matmuls -> thp, ACT copy -> cA[64:94]
  stage1(c-1): mm1 (4 MMs), ACT relus -> zA/zB f16
  stage2(c-2): mm3 + mm2rep4 (4 MMs), ACT vtmp, Pool v-accum-DMA,
               DVE t_tilde stt, ACT u = ik*ttr, DVE clip -> trep
PSUM banks: zp1,zp2 (x2 bufs) + tr + thp + vp (x2) = 8.
"""
import sys
import numpy as np

sys.path.insert(0, "/opt/trn_rl_repo")

from contextlib import ExitStack

import concourse.bass as bass
import concourse.tile as tile
from concourse import mybir
from concourse.bass_utils import run_bass_kernel_spmd

B = 16384
K = 30
LAYERS = 90
VL = 60
ZL = 240
NCORES = 8
BC = B // NCORES          # 2048
NCHUNK = 4
CH = BC // NCHUNK         # 512

F32 = mybir.dt.float32
MM_DT = mybir.dt.float32r   # fp32 data on the fast PE path (1 cyc/row at N>=256)
F16 = mybir.dt.float16

AO = mybir.AluOpType
RELU = mybir.ActivationFunctionType.Relu
IDENT = mybir.ActivationFunctionType.Identity
LAST_RESULT = None  # BassKernelResults of the most recent run (for profiling)


def build_kernel(inv_kap):
    nc = bass.Bass()

    hr_in = nc.declare_dram_parameter("HrT", [K, BC], F16, isOutput=False)
    hhy_in = nc.declare_dram_parameter("HHY", [120, 8 * BC], F16, isOutput=False)
    w1a_in = nc.declare_dram_parameter("W1A", [LAYERS, 126, ZL], F16, isOutput=False)
    w1b_in = nc.declare_dram_parameter("W1B", [LAYERS, K, ZL], F16, isOutput=False)
    w2ra_in = nc.declare_dram_parameter("W2RA", [LAYERS, 127, 120], F16, isOutput=False)
    w2rb_in = nc.declare_dram_parameter("W2RB", [LAYERS, 113, 120], F16, isOutput=False)
    w3a_in = nc.declare_dram_parameter("W3A", [LAYERS, 127, VL], F16, isOutput=False)
    w3b_in = nc.declare_dram_parameter("W3B", [LAYERS, 113, VL], F16, isOutput=False)
    bias_in = nc.declare_dram_parameter("BIAS", [LAYERS, 128, 4], F32, isOutput=False)
    sel_in = nc.declare_dram_parameter("SEL", [120, 8 * K], F16, isOutput=False)
    zero32_in = nc.declare_dram_parameter("ZERO32", [126, BC], F16, isOutput=False)
    zero16_in = nc.declare_dram_parameter("ZERO16", [120, BC], F16, isOutput=False)
    out_dram = nc.declare_dram_parameter("OUT", [LAYERS, K, BC], F16, isOutput=True)

    with tile.TileContext(nc) as tc, ExitStack() as ctx:
        persist = ctx.enter_context(tc.tile_pool(name="persist", bufs=1))
        wpool = ctx.enter_context(tc.tile_pool(name="w", bufs=3))
        zpool = ctx.enter_context(tc.tile_pool(name="z", bufs=3))
        upool = ctx.enter_context(tc.tile_pool(name="u", bufs=3))
        pp_z = ctx.enter_context(tc.tile_pool(name="ps_z", bufs=2, space="PSUM"))
        pp_t = ctx.enter_context(tc.tile_pool(name="ps_t", bufs=1, space="PSUM"))
        pp_h = ctx.enter_context(tc.tile_pool(name="ps_h", bufs=2, space="PSUM"))
        pp_v = ctx.enter_context(tc.tile_pool(name="ps_v", bufs=1, space="PSUM"))

        # ---- persistent state
        cA = persist.tile([126, BC], F16)     # v | tH | Hr
        trep = persist.tile([120, BC], F16)     # t rep4 (einsum + mm1 rhs + output)
        ttr = persist.tile([120, BC], F32)      # t_tilde rep4
        hhy = persist.tile([120, 8, BC], F16)   # HH rearranged
        pbuf = persist.tile([120, 8, BC], F16)  # einsum products
        sel = persist.tile([120, 8, K], F16)    # selector weights

        nc.gpsimd.dma_start(cA[:], zero32_in[:])
        nc.gpsimd.dma_start(trep[:], zero16_in[:])
        nc.vector.memset(ttr[:], 0.0)
        nc.gpsimd.dma_start(cA[96 : 96 + K, :], hr_in[:])
        nc.gpsimd.dma_start(hhy[:].rearrange("p a b -> p (a b)"), hhy_in[:])
        nc.gpsimd.dma_start(sel[:].rearrange("p a b -> p (a b)"), sel_in[:])

        zAs = [None] * NCHUNK
        zBs = [None] * NCHUNK

        for l in range(LAYERS):
            w1a = wpool.tile([126, ZL], F16, tag="w1a")
            w1b = wpool.tile([K, ZL], F16, tag="w1b")
            w2ra = wpool.tile([127, 120], F16, tag="w2ra")
            w2rb = wpool.tile([113, 120], F16, tag="w2rb")
            w3a = wpool.tile([127, VL], F16, tag="w3a")
            w3b = wpool.tile([113, VL], F16, tag="w3b")
            bt = wpool.tile([128, 4], F32, tag="bias")
            nc.sync.dma_start(w1a[:], w1a_in[l])
            nc.sync.dma_start(w1b[:], w1b_in[l])
            nc.sync.dma_start(w2ra[:], w2ra_in[l])
            nc.sync.dma_start(w2rb[:], w2rb_in[l])
            nc.sync.dma_start(w3a[:], w3a_in[l])
            nc.sync.dma_start(w3b[:], w3b_in[l])
            nc.sync.dma_start(bt[:], bias_in[l])
            b1a = bt[0:127, 0:1]
            b1b = bt[0:113, 1:2]
            b3 = bt[0:VL, 2:3]
            b2r = bt[0:120, 3:4]

            ik = float(inv_kap[l])

            def stage0(c):
                # einsum tH for layer l chunk c (uses t of layer l-1)
                cs = bass.ts(c, CH)
                nc.vector.tensor_tensor(
                    pbuf[:, :, cs],
                    trep[:, cs].unsqueeze(1).broadcast_to((120, 8, CH)),
                    hhy[:, :, cs],
                    op=AO.mult,
                )
                thp = pp_h.tile([94, CH], F32, tag="th")
                for g in range(8):
                    nc.tensor.matmul(
                        thp[64 : 64 + K, :],
                        sel[:, g, :],
                        pbuf[:, g, cs],
                        start=(g == 0),
                        stop=(g == 7),
                        tile_position=(0, 64),
                    )
                nc.scalar.copy(cA[64 : 64 + K, cs], thp[64 : 64 + K, :])

            def stage1(c):
                # mm1 + relus for chunk c
                cs = bass.ts(c, CH)
                zp1 = pp_z.tile([127, CH], F32, tag="z1")
                zp2 = pp_z.tile([113, CH], F32, tag="z2")
                rA = cA[:, cs]
                rB = trep[0:K, cs]
                nc.tensor.matmul(zp1[:], w1a[:, 0:127], rA, start=True, stop=False)
                nc.tensor.matmul(zp1[:], w1b[:, 0:127], rB, start=False, stop=True)
                nc.tensor.matmul(zp2[:], w1a[:, 127:240], rA, start=True, stop=False)
                nc.tensor.matmul(zp2[:], w1b[:, 127:240], rB, start=False, stop=True)
                zA = zpool.tile([127, CH], F16, tag="zA")
                zB = zpool.tile([113, CH], F16, tag="zB")
                nc.scalar.activation(zA[:], zp1[:], RELU, bias=b1a)
                nc.scalar.activation(zB[:], zp2[:], RELU, bias=b1b)
                zAs[c] = zA
                zBs[c] = zB

            def stage2(c):
                # mm3 + mm2rep4 + state updates + clip for chunk c
                cs = bass.ts(c, CH)
                zA, zB = zAs[c], zBs[c]
                vp = pp_v.tile([VL, CH], F32, tag="vp")
                nc.tensor.matmul(vp[:], w3a[:], zA[:], start=True, stop=False)
                nc.tensor.matmul(vp[:], w3b[:], zB[:], start=False, stop=True)
                tr = pp_t.tile([120, CH], F32, tag="tr")
                nc.tensor.matmul(tr[:], w2ra[:], zA[:], start=True, stop=False)
                nc.tensor.matmul(tr[:], w2rb[:], zB[:], start=False, stop=True)

                vtmp = zpool.tile([VL, CH], F16, tag="vtmp")
                nc.scalar.activation(vtmp[:], vp[:], IDENT, bias=b3)
                nc.gpsimd.dma_start(cA[0:VL, cs], vtmp[:], accum_op=AO.add)

                nc.vector.scalar_tensor_tensor(
                    ttr[:, cs], tr[:], b2r, ttr[:, cs], op0=AO.add, op1=AO.add)

                u = upool.tile([120, CH], F16, tag="u")
                nc.scalar.activation(u[:], ttr[:, cs], IDENT, scale=ik)
                nc.vector.tensor_scalar(
                    trep[:, cs], u[:], 1.0, -1.0, op0=AO.min, op1=AO.max)

            for c in range(NCHUNK + 2):
                if c < NCHUNK and l > 0:
                    stage0(c)
                if 1 <= c < NCHUNK + 1:
                    stage1(c - 1)
                if c >= 2:
                    stage2(c - 2)

            # ---- emit t of this layer (block j4=0 of trep is plain t)
            nc.sync.dma_start(out_dram[l], trep[0:K, :])

    _split_waits(nc)
    return nc


def _split_waits(nc, limit=1):
    """This toolchain build only accepts one sem-wait per instruction;
    hoist surplus waits onto same-engine NoOps inserted before the inst."""
    ctr = 0
    for f in nc.m.functions:
        for blk in f.blocks:
            insts = blk.instructions
            if not any(
                i.sync_info and i.sync_info.on_wait and len(i.sync_info.on_wait) > limit
                for i in insts
            ):
                continue
            new = []
            for inst in insts:
                si = inst.sync_info
                if si and si.on_wait and len(si.on_wait) > limit:
                    waits = list(si.on_wait)
                    extra, keep = waits[:-limit], waits[-limit:]
                    for w in extra:
                        ctr += 1
                        n = mybir.InstNoOp(name=f"WSPLIT-{ctr}", ins=[], outs=[])
                        n.engine = inst.engine
                        n.sync_info = mybir.SyncInfo(on_wait=[w], on_update=[])
                        new.append(n)
                    si.on_wait = keep
                new.append(inst)
            blk.instructions = new
    return ctr


def _prep_shared(W1, b1, W2, b2, W3, b3):
    L = W1.shape[0]
    # cA row order: [v(0:60) | pad4 | tH(64:94) | pad2 | Hr(96:126)]
    W1A = np.zeros((L, 126, ZL), np.float16)
    W1A[:, 0:VL] = W1[:, :, 30:90].transpose(0, 2, 1)        # v cols
    W1A[:, 64:94] = W1[:, :, 120:150].transpose(0, 2, 1)     # tH cols
    W1A[:, 96:126] = W1[:, :, 0:30].transpose(0, 2, 1)       # Hr cols
    W1B = np.ascontiguousarray(
        W1[:, :, 90:120].transpose(0, 2, 1)).astype(np.float16)  # t cols [L,30,240]

    # rep4 W2: out partition p=(j4*30+k) gets W2 row p%30
    W2T = W2.transpose(0, 2, 1)                              # [L, 240, 30]
    W2R = np.tile(W2T, (1, 1, 4)).astype(np.float16)
    W2RA = np.ascontiguousarray(W2R[:, 0:127])
    W2RB = np.ascontiguousarray(W2R[:, 127:240])
    W3T = W3.transpose(0, 2, 1).astype(np.float16)
    W3A = np.ascontiguousarray(W3T[:, 0:127])
    W3B = np.ascontiguousarray(W3T[:, 127:240])

    BIAS = np.zeros((L, 128, 4), np.float32)
    BIAS[:, 0:127, 0] = b1[:, 0:127]
    BIAS[:, 0:113, 1] = b1[:, 127:240]
    BIAS[:, 0:VL, 2] = b3
    BIAS[:, 0:120, 3] = np.tile(b2, (1, 4))

    # selector: thp[j] = sum_k t[k]*HH[k,j];  p = j4*30+k, group g = j//4
    SEL = np.zeros((120, 8, K), np.float16)
    for j in range(K):
        g, j4 = j // 4, j % 4
        SEL[j4 * K : (j4 + 1) * K, g, j] = 1.0
    return W1A, W1B, W2RA, W2RB, W3A, W3B, BIAS, SEL.reshape(120, 8 * K)


def kernel(Hr, HH, W1, b1, W2, b2, W3, b3, kappa):
    Hr = np.asarray(Hr, np.float32)
    HH = np.asarray(HH, np.float32)
    W1 = np.asarray(W1, np.float32)
    b1 = np.asarray(b1, np.float32)
    W2 = np.asarray(W2, np.float32)
    b2 = np.asarray(b2, np.float32)
    W3 = np.asarray(W3, np.float32)
    b3 = np.asarray(b3, np.float32)
    kappa = np.asarray(kappa, np.float32)

    W1A, W1B, W2RA, W2RB, W3A, W3B, BIAS, SELh = _prep_shared(W1, b1, W2, b2, W3, b3)
    inv_kap = (1.0 / np.abs(kappa)).astype(np.float32)

    in_maps = []
    for ci in range(NCORES):
        sl = slice(ci * BC, (ci + 1) * BC)
        HrT = np.ascontiguousarray(Hr[sl].T).astype(np.float16)
        HHp = np.zeros((BC, K, 32), np.float32)
        HHp[:, :, :K] = HH[sl]
        # HHY[p=(j4*30+k), (g, b)] = HH[b, k, g*4+j4]
        HHY = HHp.reshape(BC, K, 8, 4).transpose(3, 1, 2, 0).reshape(120, 8 * BC)
        in_maps.append({
            "HrT": HrT, "HHY": HHY.astype(np.float16),
            "W1A": W1A, "W1B": W1B, "W2RA": W2RA, "W2RB": W2RB,
            "W3A": W3A, "W3B": W3B,
            "BIAS": BIAS, "SEL": SELh,
            "ZERO32": np.zeros((126, BC), np.float16),
            "ZERO16": np.zeros((120, BC), np.float16),
        })

    nc = build_kernel(inv_kap)
    res = run_bass_kernel_spmd(nc, in_maps, list(range(NCORES)))
    global LAST_RESULT
    LAST_RESULT = res
    out = np.concatenate(
        [r["OUT"].transpose(0, 2, 1) for r in res.results], axis=1
    )
    return np.ascontiguousarray(out.astype(np.float32))


# revision 24
# speedup vs baseline: 1.2032x; 1.2032x over previous
"""DetNet Trainium2 kernel v10 (all-f16, 127/113 m-split to defeat FWL): 90-layer MLP recurrence, data-parallel over 8 cores.

Per core (2048 samples), features on partitions, batch on free axis.
Partition layouts keep every engine op same-base (non-0 bases 32-aligned,
<=32 rows):
  cA   [126,2048] f32r: v(0:60) | pad | tH(64:94) | pad | Hr(96:126)
  trep [120,2048] f16 : t replicated 4x (block j4 at partitions 30*j4+k);
       produced rep4 for free by the W2-rep4 matmul (M is free on PE)
  ttr  [120,2048] f32 : t_tilde state, rep4

Three-stage chunk pipeline per layer (keeps PE busy across the
sel->tH-copy->mm1 and mm1->relu->mm23 cross-engine dependencies):
  stage0(c):   DVE products, 8 selector matmuls -> thp, ACT copy -> cA[64:94]
  stage1(c-1): mm1 (4 MMs), ACT relus -> zA/zB f16
  stage2(c-2): mm3 + mm2rep4 (4 MMs), ACT vtmp, Pool v-accum-DMA,
               DVE t_tilde stt, ACT u = ik*ttr, DVE clip -> trep
PSUM banks: zp1,zp2 (x2 bufs) + tr + thp + vp (x2) = 8.
"""
import sys
import numpy as np

sys.path.insert(0, "/opt/trn_rl_repo")

from contextlib import ExitStack

import concourse.bass as bass
import concourse.tile as tile
from concourse import mybir
from concourse.bass_utils import run_bass_kernel_spmd

B = 16384
K = 30
LAYERS = 90
VL = 60
ZL = 240
NCORES = 8
BC = B // NCORES          # 2048
NCHUNK = 4
CH = BC // NCHUNK         # 512

F32 = mybir.dt.float32
MM_DT = mybir.dt.float32r   # fp32 data on the fast PE path (1 cyc/row at N>=256)
F16 = mybir.dt.float16

AO = mybir.AluOpType
RELU = mybir.ActivationFunctionType.Relu
IDENT = mybir.ActivationFunctionType.Identity
LAST_RESULT = None  # BassKernelResults of the most recent run (for profiling)


def build_kernel(inv_kap):
    nc = bass.Bass()

    hr_in = nc.declare_dram_parameter("HrT", [K, BC], F16, isOutput=False)
    hhy_in = nc.declare_dram_parameter("HHY", [120, 8 * BC], F16, isOutput=False)
    w1a_in = nc.declare_dram_parameter("W1A", [LAYERS, 126, ZL], F16, isOutput=False)
    w1b_in = nc.declare_dram_parameter("W1B", [LAYERS, K, ZL], F16, isOutput=False)
    w2ra_in = nc.declare_dram_parameter("W2RA", [LAYERS, 127, 120], F16, isOutput=False)
    w2rb_in = nc.declare_dram_parameter("W2RB", [LAYERS, 113, 120], F16, isOutput=False)
    w3a_in = nc.declare_dram_parameter("W3A", [LAYERS, 127, VL], F16, isOutput=False)
    w3b_in = nc.declare_dram_parameter("W3B", [LAYERS, 113, VL], F16, isOutput=False)
    bias_in = nc.declare_dram_parameter("BIAS", [LAYERS, 128, 4], F32, isOutput=False)
    sel_in = nc.declare_dram_parameter("SEL", [120, 8 * K], F16, isOutput=False)
    zero32_in = nc.declare_dram_parameter("ZERO32", [126, BC], F16, isOutput=False)
    zero16_in = nc.declare_dram_parameter("ZERO16", [120, BC], F16, isOutput=False)
    out_dram = nc.declare_dram_parameter("OUT", [LAYERS, K, BC], F16, isOutput=True)

    with tile.TileContext(nc) as tc, ExitStack() as ctx:
        persist = ctx.enter_context(tc.tile_pool(name="persist", bufs=1))
        wpool = ctx.enter_context(tc.tile_pool(name="w", bufs=3))
        zpool = ctx.enter_context(tc.tile_pool(name="z", bufs=3))
        upool = ctx.enter_context(tc.tile_pool(name="u", bufs=3))
        pp_z = ctx.enter_context(tc.tile_pool(name="ps_z", bufs=2, space="PSUM"))
        pp_t = ctx.enter_context(tc.tile_pool(name="ps_t", bufs=1, space="PSUM"))
        pp_h = ctx.enter_context(tc.tile_pool(name="ps_h", bufs=1, space="PSUM"))
        pp_v = ctx.enter_context(tc.tile_pool(name="ps_v", bufs=2, space="PSUM"))

        # ---- persistent state
        cA = persist.tile([126, BC], F16)     # v | tH | Hr
        trep = persist.tile([120, BC], F16)     # t rep4 (einsum + mm1 rhs + output)
        ttr = persist.tile([120, BC], F32)      # t_tilde rep4
        hhy = persist.tile([120, 8, BC], F16)   # HH rearranged
        pbuf = persist.tile([120, 8, BC], F16)  # einsum products
        sel = persist.tile([120, 8, K], F16)    # selector weights

        nc.gpsimd.dma_start(cA[:], zero32_in[:])
        nc.gpsimd.dma_start(trep[:], zero16_in[:])
        nc.vector.memset(ttr[:], 0.0)
        nc.gpsimd.dma_start(cA[96 : 96 + K, :], hr_in[:])
        nc.gpsimd.dma_start(hhy[:].rearrange("p a b -> p (a b)"), hhy_in[:])
        nc.gpsimd.dma_start(sel[:].rearrange("p a b -> p (a b)"), sel_in[:])

        zAs = [None] * NCHUNK
        zBs = [None] * NCHUNK

        for l in range(LAYERS):
            w1a = wpool.tile([126, ZL], F16, tag="w1a")
            w1b = wpool.tile([K, ZL], F16, tag="w1b")
            w2ra = wpool.tile([127, 120], F16, tag="w2ra")
            w2rb = wpool.tile([113, 120], F16, tag="w2rb")
            w3a = wpool.tile([127, VL], F16, tag="w3a")
            w3b = wpool.tile([113, VL], F16, tag="w3b")
            bt = wpool.tile([128, 4], F32, tag="bias")
            nc.sync.dma_start(w1a[:], w1a_in[l])
            nc.sync.dma_start(w1b[:], w1b_in[l])
            nc.sync.dma_start(w2ra[:], w2ra_in[l])
            nc.sync.dma_start(w2rb[:], w2rb_in[l])
            nc.sync.dma_start(w3a[:], w3a_in[l])
            nc.sync.dma_start(w3b[:], w3b_in[l])
            nc.sync.dma_start(bt[:], bias_in[l])
            b1a = bt[0:127, 0:1]
            b1b = bt[0:113, 1:2]
            b3 = bt[0:VL, 2:3]
            b2r = bt[0:120, 3:4]

            ik = float(inv_kap[l])

            def stage0(c):
                # einsum tH for layer l chunk c (uses t of layer l-1)
                cs = bass.ts(c, CH)
                nc.vector.tensor_tensor(
                    pbuf[:, :, cs],
                    trep[:, cs].unsqueeze(1).broadcast_to((120, 8, CH)),
                    hhy[:, :, cs],
                    op=AO.mult,
                )
                thp = pp_h.tile([94, CH], F32, tag="th")
                for g in range(8):
                    nc.tensor.matmul(
                        thp[64 : 64 + K, :],
                        sel[:, g, :],
                        pbuf[:, g, cs],
                        start=(g == 0),
                        stop=(g == 7),
                        tile_position=(0, 64),
                    )
                nc.scalar.copy(cA[64 : 64 + K, cs], thp[64 : 64 + K, :])

            def stage1(c):
                # mm1 + relus for chunk c
                cs = bass.ts(c, CH)
                zp1 = pp_z.tile([127, CH], F32, tag="z1")
                zp2 = pp_z.tile([113, CH], F32, tag="z2")
                rA = cA[:, cs]
                rB = trep[0:K, cs]
                nc.tensor.matmul(zp1[:], w1a[:, 0:127], rA, start=True, stop=False)
                nc.tensor.matmul(zp1[:], w1b[:, 0:127], rB, start=False, stop=True)
                nc.tensor.matmul(zp2[:], w1a[:, 127:240], rA, start=True, stop=False)
                nc.tensor.matmul(zp2[:], w1b[:, 127:240], rB, start=False, stop=True)
                zA = zpool.tile([127, CH], F16, tag="zA")
                zB = zpool.tile([113, CH], F16, tag="zB")
                nc.scalar.activation(zA[:], zp1[:], RELU, bias=b1a)
                nc.scalar.activation(zB[:], zp2[:], RELU, bias=b1b)
                zAs[c] = zA
                zBs[c] = zB

            def stage2(c):
                # mm3 + mm2rep4 + state updates + clip for chunk c
                cs = bass.ts(c, CH)
                zA, zB = zAs[c], zBs[c]
                vp = pp_v.tile([VL, CH], F32, tag="vp")
                nc.tensor.matmul(vp[:], w3a[:], zA[:], start=True, stop=False)
                nc.tensor.matmul(vp[:], w3b[:], zB[:], start=False, stop=True)
                tr = pp_t.tile([120, CH], F32, tag="tr")
                nc.tensor.matmul(tr[:], w2ra[:], zA[:], start=True, stop=False)
                nc.tensor.matmul(tr[:], w2rb[:], zB[:], start=False, stop=True)

                vtmp = zpool.tile([VL, CH], F16, tag="vtmp")
                nc.scalar.activation(vtmp[:], vp[:], IDENT, bias=b3)
                nc.gpsimd.dma_start(cA[0:VL, cs], vtmp[:], accum_op=AO.add)

                nc.vector.scalar_tensor_tensor(
                    ttr[:, cs], tr[:], b2r, ttr[:, cs], op0=AO.add, op1=AO.add)

                u = upool.tile([120, CH], F16, tag="u")
                nc.scalar.activation(u[:], ttr[:, cs], IDENT, scale=ik)
                nc.vector.tensor_scalar(
                    trep[:, cs], u[:], 1.0, -1.0, op0=AO.min, op1=AO.max)

            for c in range(NCHUNK + 2):
                if c < NCHUNK and l > 0:
                    stage0(c)
                if 1 <= c < NCHUNK + 1:
                    stage1(c - 1)
                if c >= 2:
                    stage2(c - 2)

            # ---- emit t of this layer (block j4=0 of trep is plain t)
            nc.sync.dma_start(out_dram[l], trep[0:K, :])

    _split_waits(nc)
    return nc


def _split_waits(nc, limit=1):
    """This toolchain build only accepts one sem-wait per instruction;
    hoist surplus waits onto same-engine NoOps inserted before the inst."""
    ctr = 0
    for f in nc.m.functions:
        for blk in f.blocks:
            insts = blk.instructions
            if not any(
                i.sync_info and i.sync_info.on_wait and len(i.sync_info.on_wait) > limit
                for i in insts
            ):
                continue
            new = []
            for inst in insts:
                si = inst.sync_info
                if si and si.on_wait and len(si.on_wait) > limit:
                    waits = list(si.on_wait)
                    extra, keep = waits[:-limit], waits[-limit:]
                    for w in extra:
                        ctr += 1
                        n = mybir.InstNoOp(name=f"WSPLIT-{ctr}", ins=[], outs=[])
                        n.engine = inst.engine
                        n.sync_info = mybir.SyncInfo(on_wait=[w], on_update=[])
                        new.append(n)
                    si.on_wait = keep
                new.append(inst)
            blk.instructions = new
    return ctr


def _prep_shared(W1, b1, W2, b2, W3, b3):
    L = W1.shape[0]
    # cA row order: [v(0:60) | pad4 | tH(64:94) | pad2 | Hr(96:126)]
    W1A = np.zeros((L, 126, ZL), np.float16)
    W1A[:, 0:VL] = W1[:, :, 30:90].transpose(0, 2, 1)        # v cols
    W1A[:, 64:94] = W1[:, :, 120:150].transpose(0, 2, 1)     # tH cols
    W1A[:, 96:126] = W1[:, :, 0:30].transpose(0, 2, 1)       # Hr cols
    W1B = np.ascontiguousarray(
        W1[:, :, 90:120].transpose(0, 2, 1)).astype(np.float16)  # t cols [L,30,240]

    # rep4 W2: out partition p=(j4*30+k) gets W2 row p%30
    W2T = W2.transpose(0, 2, 1)                              # [L, 240, 30]
    W2R = np.tile(W2T, (1, 1, 4)).astype(np.float16)
    W2RA = np.ascontiguousarray(W2R[:, 0:127])
    W2RB = np.ascontiguousarray(W2R[:, 127:240])
    W3T = W3.transpose(0, 2, 1).astype(np.float16)
    W3A = np.ascontiguousarray(W3T[:, 0:127])
    W3B = np.ascontiguousarray(W3T[:, 127:240])

    BIAS = np.zeros((L, 128, 4), np.float32)
    BIAS[:, 0:127, 0] = b1[:, 0:127]
    BIAS[:, 0:113, 1] = b1[:, 127:240]
    BIAS[:, 0:VL, 2] = b3
    BIAS[:, 0:120, 3] = np.tile(b2, (1, 4))

    # selector: thp[j] = sum_k t[k]*HH[k,j];  p = j4*30+k, group g = j//4
    SEL = np.zeros((120, 8, K), np.float16)
    for j in range(K):
        g, j4 = j // 4, j % 4
        SEL[j4 * K : (j4 + 1) * K, g, j] = 1.0
    return W1A, W1B, W2RA, W2RB, W3A, W3B, BIAS, SEL.reshape(120, 8 * K)


def kernel(Hr, HH, W1, b1, W2, b2, W3, b3, kappa):
    Hr = np.asarray(Hr, np.float32)
    HH = np.asarray(HH, np.float32)
    W1 = np.asarray(W1, np.float32)
    b1 = np.asarray(b1, np.float32)
    W2 = np.asarray(W2, np.float32)
    b2 = np.asarray(b2, np.float32)
    W3 = np.asarray(W3, np.float32)
    b3 = np.asarray(b3, np.float32)
    kappa = np.asarray(kappa, np.float32)

    W1A, W1B, W2RA, W2RB, W3A, W3B, BIAS, SELh = _prep_shared(W1, b1, W2, b2, W3, b3)
    inv_kap = (1.0 / np.abs(kappa)).astype(np.float32)

    in_maps = []
    for ci in range(NCORES):
        sl = slice(ci * BC, (ci + 1) * BC)
        HrT = np.ascontiguousarray(Hr[sl].T).astype(np.float16)
        HHp = np.zeros((BC, K, 32), np.float32)
        HHp[:, :, :K] = HH[sl]
        # HHY[p=(j4*30+k), (g, b)] = HH[b, k, g*4+j4]
        HHY = HHp.reshape(BC, K, 8, 4).transpose(3, 1, 2, 0).reshape(120, 8 * BC)
        in_maps.append({
            "HrT": HrT, "HHY": HHY.astype(np.float16),
            "W1A": W1A, "W1B": W1B, "W2RA": W2RA, "W2RB": W2RB,
            "W3A": W3A, "W3B": W3B,
            "BIAS": BIAS, "SEL": SELh,
            "ZERO32": np.zeros((126, BC), np.float16),
            "ZERO16": np.zeros((120, BC), np.float16),
        })

    nc = build_kernel(inv_kap)
    res = run_bass_kernel_spmd(nc, in_maps, list(range(NCORES)))
    global LAST_RESULT
    LAST_RESULT = res
    out = np.concatenate(
        [r["OUT"].transpose(0, 2, 1) for r in res.results], axis=1
    )
    return np.ascontiguousarray(out.astype(np.float32))


# revision 25
# speedup vs baseline: 1.2417x; 1.0320x over previous
"""DetNet Trainium2 kernel v10 (all-f16, 127/113 m-split to defeat FWL): 90-layer MLP recurrence, data-parallel over 8 cores.

Per core (2048 samples), features on partitions, batch on free axis.
Partition layouts keep every engine op same-base (non-0 bases 32-aligned,
<=32 rows):
  cA   [126,2048] f32r: v(0:60) | pad | tH(64:94) | pad | Hr(96:126)
  trep [120,2048] f16 : t replicated 4x (block j4 at partitions 30*j4+k);
       produced rep4 for free by the W2-rep4 matmul (M is free on PE)
  ttr  [120,2048] f32 : t_tilde state, rep4

Three-stage chunk pipeline per layer (keeps PE busy across the
sel->tH-copy->mm1 and mm1->relu->mm23 cross-engine dependencies):
  stage0(c):   DVE products, 8 selector matmuls -> thp, ACT copy -> cA[64:94]
  stage1(c-1): mm1 (4 MMs), ACT relus -> zA/zB f16
  stage2(c-2): mm3 + mm2rep4 (4 MMs), ACT vtmp, Pool v-accum-DMA,
               DVE t_tilde stt, ACT u = ik*ttr, DVE clip -> trep
PSUM banks: zp1,zp2 (x2 bufs) + tr + thp + vp (x2) = 8.
"""
import sys
import numpy as np

sys.path.insert(0, "/opt/trn_rl_repo")

from contextlib import ExitStack

import concourse.bass as bass
import concourse.tile as tile
from concourse import mybir
from concourse.bass_utils import run_bass_kernel_spmd

B = 16384
K = 30
LAYERS = 90
VL = 60
ZL = 240
NCORES = 8
BC = B // NCORES          # 2048
NCHUNK = 4
CH = BC // NCHUNK         # 512

F32 = mybir.dt.float32
MM_DT = mybir.dt.float32r   # fp32 data on the fast PE path (1 cyc/row at N>=256)
F16 = mybir.dt.float16

AO = mybir.AluOpType
RELU = mybir.ActivationFunctionType.Relu
IDENT = mybir.ActivationFunctionType.Identity
LAST_RESULT = None  # BassKernelResults of the most recent run (for profiling)


def build_kernel(inv_kap):
    nc = bass.Bass()

    hr_in = nc.declare_dram_parameter("HrT", [K, BC], F16, isOutput=False)
    hhy_in = nc.declare_dram_parameter("HHY", [120, 8 * BC], F16, isOutput=False)
    w1a_in = nc.declare_dram_parameter("W1A", [LAYERS, 126, ZL], F16, isOutput=False)
    w1b_in = nc.declare_dram_parameter("W1B", [LAYERS, K, ZL], F16, isOutput=False)
    w2ra_in = nc.declare_dram_parameter("W2RA", [LAYERS, 127, 120], F16, isOutput=False)
    w2rb_in = nc.declare_dram_parameter("W2RB", [LAYERS, 113, 120], F16, isOutput=False)
    w3a_in = nc.declare_dram_parameter("W3A", [LAYERS, 127, VL], F16, isOutput=False)
    w3b_in = nc.declare_dram_parameter("W3B", [LAYERS, 113, VL], F16, isOutput=False)
    bias_in = nc.declare_dram_parameter("BIAS", [LAYERS, 128, 4], F32, isOutput=False)
    sel_in = nc.declare_dram_parameter("SEL", [120, 8 * K], F16, isOutput=False)
    zero32_in = nc.declare_dram_parameter("ZERO32", [126, BC], F16, isOutput=False)
    zero16_in = nc.declare_dram_parameter("ZERO16", [120, BC], F16, isOutput=False)
    out_dram = nc.declare_dram_parameter("OUT", [LAYERS, K, BC], F16, isOutput=True)

    with tile.TileContext(nc) as tc, ExitStack() as ctx:
        persist = ctx.enter_context(tc.tile_pool(name="persist", bufs=1))
        wpool = ctx.enter_context(tc.tile_pool(name="w", bufs=3))
        zpool = ctx.enter_context(tc.tile_pool(name="z", bufs=3))
        upool = ctx.enter_context(tc.tile_pool(name="u", bufs=3))
        pp_z = ctx.enter_context(tc.tile_pool(name="ps_z", bufs=1, space="PSUM"))
        pp_t = ctx.enter_context(tc.tile_pool(name="ps_t", bufs=2, space="PSUM"))
        pp_h = ctx.enter_context(tc.tile_pool(name="ps_h", bufs=2, space="PSUM"))
        pp_v = ctx.enter_context(tc.tile_pool(name="ps_v", bufs=2, space="PSUM"))

        # ---- persistent state
        cA = persist.tile([126, BC], F16)     # v | tH | Hr
        trep = persist.tile([120, BC], F16)     # t rep4 (einsum + mm1 rhs + output)
        ttr = persist.tile([120, BC], F32)      # t_tilde rep4
        hhy = persist.tile([120, 8, BC], F16)   # HH rearranged
        pbuf = persist.tile([120, 8, BC], F16)  # einsum products
        sel = persist.tile([120, 8, K], F16)    # selector weights

        nc.gpsimd.dma_start(cA[:], zero32_in[:])
        nc.gpsimd.dma_start(trep[:], zero16_in[:])
        nc.vector.memset(ttr[:], 0.0)
        nc.gpsimd.dma_start(cA[96 : 96 + K, :], hr_in[:])
        nc.gpsimd.dma_start(hhy[:].rearrange("p a b -> p (a b)"), hhy_in[:])
        nc.gpsimd.dma_start(sel[:].rearrange("p a b -> p (a b)"), sel_in[:])

        zAs = [None] * NCHUNK
        zBs = [None] * NCHUNK

        for l in range(LAYERS):
            w1a = wpool.tile([126, ZL], F16, tag="w1a")
            w1b = wpool.tile([K, ZL], F16, tag="w1b")
            w2ra = wpool.tile([127, 120], F16, tag="w2ra")
            w2rb = wpool.tile([113, 120], F16, tag="w2rb")
            w3a = wpool.tile([127, VL], F16, tag="w3a")
            w3b = wpool.tile([113, VL], F16, tag="w3b")
            bt = wpool.tile([128, 4], F32, tag="bias")
            nc.sync.dma_start(w1a[:], w1a_in[l])
            nc.sync.dma_start(w1b[:], w1b_in[l])
            nc.sync.dma_start(w2ra[:], w2ra_in[l])
            nc.sync.dma_start(w2rb[:], w2rb_in[l])
            nc.sync.dma_start(w3a[:], w3a_in[l])
            nc.sync.dma_start(w3b[:], w3b_in[l])
            nc.sync.dma_start(bt[:], bias_in[l])
            b1a = bt[0:127, 0:1]
            b1b = bt[0:113, 1:2]
            b3 = bt[0:VL, 2:3]
            b2r = bt[0:120, 3:4]

            ik = float(inv_kap[l])

            def stage0(c):
                # einsum tH for layer l chunk c (uses t of layer l-1)
                cs = bass.ts(c, CH)
                nc.vector.tensor_tensor(
                    pbuf[:, :, cs],
                    trep[:, cs].unsqueeze(1).broadcast_to((120, 8, CH)),
                    hhy[:, :, cs],
                    op=AO.mult,
                )
                thp = pp_h.tile([94, CH], F32, tag="th")
                for g in range(8):
                    nc.tensor.matmul(
                        thp[64 : 64 + K, :],
                        sel[:, g, :],
                        pbuf[:, g, cs],
                        start=(g == 0),
                        stop=(g == 7),
                        tile_position=(0, 64),
                    )
                nc.scalar.copy(cA[64 : 64 + K, cs], thp[64 : 64 + K, :])

            def stage1(c):
                # mm1 + relus for chunk c
                cs = bass.ts(c, CH)
                zp1 = pp_z.tile([127, CH], F32, tag="z1")
                zp2 = pp_z.tile([113, CH], F32, tag="z2")
                rA = cA[:, cs]
                rB = trep[0:K, cs]
                nc.tensor.matmul(zp1[:], w1a[:, 0:127], rA, start=True, stop=False)
                nc.tensor.matmul(zp1[:], w1b[:, 0:127], rB, start=False, stop=True)
                nc.tensor.matmul(zp2[:], w1a[:, 127:240], rA, start=True, stop=False)
                nc.tensor.matmul(zp2[:], w1b[:, 127:240], rB, start=False, stop=True)
                zA = zpool.tile([127, CH], F16, tag="zA")
                zB = zpool.tile([113, CH], F16, tag="zB")
                nc.scalar.activation(zA[:], zp1[:], RELU, bias=b1a)
                nc.scalar.activation(zB[:], zp2[:], RELU, bias=b1b)
                zAs[c] = zA
                zBs[c] = zB

            def stage2(c):
                # mm3 + mm2rep4 + state updates + clip for chunk c
                cs = bass.ts(c, CH)
                zA, zB = zAs[c], zBs[c]
                vp = pp_v.tile([VL, CH], F32, tag="vp")
                nc.tensor.matmul(vp[:], w3a[:], zA[:], start=True, stop=False)
                nc.tensor.matmul(vp[:], w3b[:], zB[:], start=False, stop=True)
                tr = pp_t.tile([120, CH], F32, tag="tr")
                nc.tensor.matmul(tr[:], w2ra[:], zA[:], start=True, stop=False)
                nc.tensor.matmul(tr[:], w2rb[:], zB[:], start=False, stop=True)

                vtmp = zpool.tile([VL, CH], F16, tag="vtmp")
                nc.scalar.activation(vtmp[:], vp[:], IDENT, bias=b3)
                nc.gpsimd.dma_start(cA[0:VL, cs], vtmp[:], accum_op=AO.add)

                nc.vector.scalar_tensor_tensor(
                    ttr[:, cs], tr[:], b2r, ttr[:, cs], op0=AO.add, op1=AO.add)

                u = upool.tile([120, CH], F16, tag="u")
                nc.scalar.activation(u[:], ttr[:, cs], IDENT, scale=ik)
                nc.vector.tensor_scalar(
                    trep[:, cs], u[:], 1.0, -1.0, op0=AO.min, op1=AO.max)

            for c in range(NCHUNK + 2):
                if c < NCHUNK and l > 0:
                    stage0(c)
                if 1 <= c < NCHUNK + 1:
                    stage1(c - 1)
                if c >= 2:
                    stage2(c - 2)

            # ---- emit t of this layer (block j4=0 of trep is plain t)
            nc.sync.dma_start(out_dram[l], trep[0:K, :])

    _split_waits(nc)
    return nc


def _split_waits(nc, limit=1):
    """This toolchain build only accepts one sem-wait per instruction;
    hoist surplus waits onto same-engine NoOps inserted before the inst."""
    ctr = 0
    for f in nc.m.functions:
        for blk in f.blocks:
            insts = blk.instructions
            if not any(
                i.sync_info and i.sync_info.on_wait and len(i.sync_info.on_wait) > limit
                for i in insts
            ):
                continue
            new = []
            for inst in insts:
                si = inst.sync_info
                if si and si.on_wait and len(si.on_wait) > limit:
                    waits = list(si.on_wait)
                    extra, keep = waits[:-limit], waits[-limit:]
                    for w in extra:
                        ctr += 1
                        n = mybir.InstNoOp(name=f"WSPLIT-{ctr}", ins=[], outs=[])
                        n.engine = inst.engine
                        n.sync_info = mybir.SyncInfo(on_wait=[w], on_update=[])
                        new.append(n)
                    si.on_wait = keep
                new.append(inst)
            blk.instructions = new
    return ctr


def _prep_shared(W1, b1, W2, b2, W3, b3):
    L = W1.shape[0]
    # cA row order: [v(0:60) | pad4 | tH(64:94) | pad2 | Hr(96:126)]
    W1A = np.zeros((L, 126, ZL), np.float16)
    W1A[:, 0:VL] = W1[:, :, 30:90].transpose(0, 2, 1)        # v cols
    W1A[:, 64:94] = W1[:, :, 120:150].transpose(0, 2, 1)     # tH cols
    W1A[:, 96:126] = W1[:, :, 0:30].transpose(0, 2, 1)       # Hr cols
    W1B = np.ascontiguousarray(
        W1[:, :, 90:120].transpose(0, 2, 1)).astype(np.float16)  # t cols [L,30,240]

    # rep4 W2: out partition p=(j4*30+k) gets W2 row p%30
    W2T = W2.transpose(0, 2, 1)                              # [L, 240, 30]
    W2R = np.tile(W2T, (1, 1, 4)).astype(np.float16)
    W2RA = np.ascontiguousarray(W2R[:, 0:127])
    W2RB = np.ascontiguousarray(W2R[:, 127:240])
    W3T = W3.transpose(0, 2, 1).astype(np.float16)
    W3A = np.ascontiguousarray(W3T[:, 0:127])
    W3B = np.ascontiguousarray(W3T[:, 127:240])

    BIAS = np.zeros((L, 128, 4), np.float32)
    BIAS[:, 0:127, 0] = b1[:, 0:127]
    BIAS[:, 0:113, 1] = b1[:, 127:240]
    BIAS[:, 0:VL, 2] = b3
    BIAS[:, 0:120, 3] = np.tile(b2, (1, 4))

    # selector: thp[j] = sum_k t[k]*HH[k,j];  p = j4*30+k, group g = j//4
    SEL = np.zeros((120, 8, K), np.float16)
    for j in range(K):
        g, j4 = j // 4, j % 4
        SEL[j4 * K : (j4 + 1) * K, g, j] = 1.0
    return W1A, W1B, W2RA, W2RB, W3A, W3B, BIAS, SEL.reshape(120, 8 * K)


def kernel(Hr, HH, W1, b1, W2, b2, W3, b3, kappa):
    Hr = np.asarray(Hr, np.float32)
    HH = np.asarray(HH, np.float32)
    W1 = np.asarray(W1, np.float32)
    b1 = np.asarray(b1, np.float32)
    W2 = np.asarray(W2, np.float32)
    b2 = np.asarray(b2, np.float32)
    W3 = np.asarray(W3, np.float32)
    b3 = np.asarray(b3, np.float32)
    kappa = np.asarray(kappa, np.float32)

    W1A, W1B, W2RA, W2RB, W3A, W3B, BIAS, SELh = _prep_shared(W1, b1, W2, b2, W3, b3)
    inv_kap = (1.0 / np.abs(kappa)).astype(np.float32)

    in_maps = []
    for ci in range(NCORES):
        sl = slice(ci * BC, (ci + 1) * BC)
        HrT = np.ascontiguousarray(Hr[sl].T).astype(np.float16)
        HHp = np.zeros((BC, K, 32), np.float32)
        HHp[:, :, :K] = HH[sl]
        # HHY[p=(j4*30+k), (g, b)] = HH[b, k, g*4+j4]
        HHY = HHp.reshape(BC, K, 8, 4).transpose(3, 1, 2, 0).reshape(120, 8 * BC)
        in_maps.append({
            "HrT": HrT, "HHY": HHY.astype(np.float16),
            "W1A": W1A, "W1B": W1B, "W2RA": W2RA, "W2RB": W2RB,
            "W3A": W3A, "W3B": W3B,
            "BIAS": BIAS, "SEL": SELh,
            "ZERO32": np.zeros((126, BC), np.float16),
            "ZERO16": np.zeros((120, BC), np.float16),
        })

    nc = build_kernel(inv_kap)
    res = run_bass_kernel_spmd(nc, in_maps, list(range(NCORES)))
    global LAST_RESULT
    LAST_RESULT = res
    out = np.concatenate(
        [r["OUT"].transpose(0, 2, 1) for r in res.results], axis=1
    )
    return np.ascontiguousarray(out.astype(np.float32))


# revision 26
# speedup vs baseline: 1.2451x; 1.0028x over previous
"""DetNet Trainium2 kernel v13 (v12 + product lookahead, zp2-first mm1 order): 90-layer MLP recurrence, data-parallel over 8 cores.

Per core (2048 samples), features on partitions, batch on free axis.
Partition layouts keep every engine op same-base (non-0 bases 32-aligned,
<=32 rows):
  cA   [126,2048] f32r: v(0:60) | pad | tH(64:94) | pad | Hr(96:126)
  trep [120,2048] f16 : t replicated 4x (block j4 at partitions 30*j4+k);
       produced rep4 for free by the W2-rep4 matmul (M is free on PE)
  ttr  [120,2048] f32 : t_tilde state, rep4

Three-stage chunk pipeline per layer (keeps PE busy across the
sel->tH-copy->mm1 and mm1->relu->mm23 cross-engine dependencies):
  stage0(c):   DVE products, 8 selector matmuls -> thp, ACT copy -> cA[64:94]
  stage1(c-1): mm1 (4 MMs), ACT relus -> zA/zB f16
  stage2(c-2): mm3 + mm2rep4 (4 MMs), ACT vtmp, Pool v-accum-DMA,
               DVE t_tilde stt, ACT u = ik*ttr, DVE clip -> trep
PSUM banks: zp1,zp2 (x2 bufs) + tr + thp + vp (x2) = 8.
"""
import sys
import numpy as np

sys.path.insert(0, "/opt/trn_rl_repo")

from contextlib import ExitStack

import concourse.bass as bass
import concourse.tile as tile
from concourse import mybir
from concourse.bass_utils import run_bass_kernel_spmd

B = 16384
K = 30
LAYERS = 90
VL = 60
ZL = 240
NCORES = 8
BC = B // NCORES          # 2048
NCHUNK = 4
CH = BC // NCHUNK         # 512

F32 = mybir.dt.float32
MM_DT = mybir.dt.float32r   # fp32 data on the fast PE path (1 cyc/row at N>=256)
F16 = mybir.dt.float16

AO = mybir.AluOpType
RELU = mybir.ActivationFunctionType.Relu
IDENT = mybir.ActivationFunctionType.Identity
LAST_RESULT = None  # BassKernelResults of the most recent run (for profiling)


def build_kernel(inv_kap):
    nc = bass.Bass()

    hr_in = nc.declare_dram_parameter("HrT", [K, BC], F16, isOutput=False)
    hhy_in = nc.declare_dram_parameter("HHY", [120, 8 * BC], F16, isOutput=False)
    w1a_in = nc.declare_dram_parameter("W1A", [LAYERS, 126, ZL], F16, isOutput=False)
    w1b_in = nc.declare_dram_parameter("W1B", [LAYERS, K, ZL], F16, isOutput=False)
    w2ra_in = nc.declare_dram_parameter("W2RA", [LAYERS, 127, 120], F16, isOutput=False)
    w2rb_in = nc.declare_dram_parameter("W2RB", [LAYERS, 113, 120], F16, isOutput=False)
    w3a_in = nc.declare_dram_parameter("W3A", [LAYERS, 127, VL], F16, isOutput=False)
    w3b_in = nc.declare_dram_parameter("W3B", [LAYERS, 113, VL], F16, isOutput=False)
    bias_in = nc.declare_dram_parameter("BIAS", [LAYERS, 128, 4], F32, isOutput=False)
    sel_in = nc.declare_dram_parameter("SEL", [120, 8 * K], F16, isOutput=False)
    zero32_in = nc.declare_dram_parameter("ZERO32", [126, BC], F16, isOutput=False)
    zero16_in = nc.declare_dram_parameter("ZERO16", [120, BC], F16, isOutput=False)
    out_dram = nc.declare_dram_parameter("OUT", [LAYERS, K, BC], F16, isOutput=True)

    with tile.TileContext(nc) as tc, ExitStack() as ctx:
        persist = ctx.enter_context(tc.tile_pool(name="persist", bufs=1))
        wpool = ctx.enter_context(tc.tile_pool(name="w", bufs=3))
        zpool = ctx.enter_context(tc.tile_pool(name="z", bufs=3))
        upool = ctx.enter_context(tc.tile_pool(name="u", bufs=3))
        pp_z = ctx.enter_context(tc.tile_pool(name="ps_z", bufs=1, space="PSUM"))
        pp_t = ctx.enter_context(tc.tile_pool(name="ps_t", bufs=2, space="PSUM"))
        pp_h = ctx.enter_context(tc.tile_pool(name="ps_h", bufs=2, space="PSUM"))
        pp_v = ctx.enter_context(tc.tile_pool(name="ps_v", bufs=2, space="PSUM"))

        # ---- persistent state
        cA = persist.tile([126, BC], F16)     # v | tH | Hr
        trep = persist.tile([120, BC], F16)     # t rep4 (einsum + mm1 rhs + output)
        ttr = persist.tile([120, BC], F32)      # t_tilde rep4
        hhy = persist.tile([120, 8, BC], F16)   # HH rearranged
        pbuf = persist.tile([120, 8, BC], F16)  # einsum products
        sel = persist.tile([120, 8, K], F16)    # selector weights

        nc.gpsimd.dma_start(cA[:], zero32_in[:])
        nc.gpsimd.dma_start(trep[:], zero16_in[:])
        nc.vector.memset(ttr[:], 0.0)
        nc.gpsimd.dma_start(cA[96 : 96 + K, :], hr_in[:])
        nc.gpsimd.dma_start(hhy[:].rearrange("p a b -> p (a b)"), hhy_in[:])
        nc.gpsimd.dma_start(sel[:].rearrange("p a b -> p (a b)"), sel_in[:])

        zAs = [None] * NCHUNK
        zBs = [None] * NCHUNK

        for l in range(LAYERS):
            w1a = wpool.tile([126, ZL], F16, tag="w1a")
            w1b = wpool.tile([K, ZL], F16, tag="w1b")
            w2ra = wpool.tile([127, 120], F16, tag="w2ra")
            w2rb = wpool.tile([113, 120], F16, tag="w2rb")
            w3a = wpool.tile([127, VL], F16, tag="w3a")
            w3b = wpool.tile([113, VL], F16, tag="w3b")
            bt = wpool.tile([128, 4], F32, tag="bias")
            nc.sync.dma_start(w1a[:], w1a_in[l])
            nc.sync.dma_start(w1b[:], w1b_in[l])
            nc.sync.dma_start(w2ra[:], w2ra_in[l])
            nc.sync.dma_start(w2rb[:], w2rb_in[l])
            nc.sync.dma_start(w3a[:], w3a_in[l])
            nc.sync.dma_start(w3b[:], w3b_in[l])
            nc.sync.dma_start(bt[:], bias_in[l])
            b1a = bt[0:127, 0:1]
            b1b = bt[0:113, 1:2]
            b3 = bt[0:VL, 2:3]
            b2r = bt[0:120, 3:4]

            ik = float(inv_kap[l])

            def stage0_products(c):
                # einsum products for chunk c (emitted one iteration early so
                # the DVE finishes before the selector matmuls need them)
                cs = bass.ts(c, CH)
                nc.vector.tensor_tensor(
                    pbuf[:, :, cs],
                    trep[:, cs].unsqueeze(1).broadcast_to((120, 8, CH)),
                    hhy[:, :, cs],
                    op=AO.mult,
                )

            def stage0_sels(c):
                # selector matmuls accumulating tH for chunk c
                cs = bass.ts(c, CH)
                thp = pp_h.tile([94, CH], F32, tag="th")
                for g in range(8):
                    nc.tensor.matmul(
                        thp[64 : 64 + K, :],
                        sel[:, g, :],
                        pbuf[:, g, cs],
                        start=(g == 0),
                        stop=(g == 7),
                        tile_position=(0, 64),
                    )
                nc.scalar.copy(cA[64 : 64 + K, cs], thp[64 : 64 + K, :])

            def stage1(c):
                # mm1 + relus for chunk c
                cs = bass.ts(c, CH)
                zp1 = pp_z.tile([127, CH], F32, tag="z1")
                zp2 = pp_z.tile([113, CH], F32, tag="z2")
                rA = cA[:, cs]
                rB = trep[0:K, cs]
                nc.tensor.matmul(zp2[:], w1a[:, 127:240], rA, start=True, stop=False)
                nc.tensor.matmul(zp2[:], w1b[:, 127:240], rB, start=False, stop=True)
                nc.tensor.matmul(zp1[:], w1a[:, 0:127], rA, start=True, stop=False)
                nc.tensor.matmul(zp1[:], w1b[:, 0:127], rB, start=False, stop=True)
                zA = zpool.tile([127, CH], F16, tag="zA")
                zB = zpool.tile([113, CH], F16, tag="zB")
                nc.scalar.activation(zB[:], zp2[:], RELU, bias=b1b)
                nc.scalar.activation(zA[:], zp1[:], RELU, bias=b1a)
                zAs[c] = zA
                zBs[c] = zB

            def stage2(c):
                # mm3 + mm2rep4 + state updates + clip for chunk c
                cs = bass.ts(c, CH)
                zA, zB = zAs[c], zBs[c]
                vp = pp_v.tile([VL, CH], F32, tag="vp")
                nc.tensor.matmul(vp[:], w3a[:], zA[:], start=True, stop=False)
                nc.tensor.matmul(vp[:], w3b[:], zB[:], start=False, stop=True)
                tr = pp_t.tile([120, CH], F32, tag="tr")
                nc.tensor.matmul(tr[:], w2ra[:], zA[:], start=True, stop=False)
                nc.tensor.matmul(tr[:], w2rb[:], zB[:], start=False, stop=True)

                vtmp = zpool.tile([VL, CH], F16, tag="vtmp")
                nc.scalar.activation(vtmp[:], vp[:], IDENT, bias=b3)
                nc.gpsimd.dma_start(cA[0:VL, cs], vtmp[:], accum_op=AO.add)

                nc.vector.scalar_tensor_tensor(
                    ttr[:, cs], tr[:], b2r, ttr[:, cs], op0=AO.add, op1=AO.add)

                u = upool.tile([120, CH], F16, tag="u")
                nc.scalar.activation(u[:], ttr[:, cs], IDENT, scale=ik)
                nc.vector.tensor_scalar(
                    trep[:, cs], u[:], 1.0, -1.0, op0=AO.min, op1=AO.max)

            if l > 0:
                stage0_products(0)
            for c in range(NCHUNK + 2):
                if c < NCHUNK and l > 0:
                    stage0_sels(c)
                if 1 <= c < NCHUNK + 1:
                    stage1(c - 1)
                if c >= 2:
                    stage2(c - 2)
                if l > 0 and c + 1 < NCHUNK:
                    stage0_products(c + 1)

            # ---- emit t of this layer (block j4=0 of trep is plain t)
            nc.sync.dma_start(out_dram[l], trep[0:K, :])

    _split_waits(nc)
    return nc


def _split_waits(nc, limit=1):
    """This toolchain build only accepts one sem-wait per instruction;
    hoist surplus waits onto same-engine NoOps inserted before the inst."""
    ctr = 0
    for f in nc.m.functions:
        for blk in f.blocks:
            insts = blk.instructions
            if not any(
                i.sync_info and i.sync_info.on_wait and len(i.sync_info.on_wait) > limit
                for i in insts
            ):
                continue
            new = []
            for inst in insts:
                si = inst.sync_info
                if si and si.on_wait and len(si.on_wait) > limit:
                    waits = list(si.on_wait)
                    extra, keep = waits[:-limit], waits[-limit:]
                    for w in extra:
                        ctr += 1
                        n = mybir.InstNoOp(name=f"WSPLIT-{ctr}", ins=[], outs=[])
                        n.engine = inst.engine
                        n.sync_info = mybir.SyncInfo(on_wait=[w], on_update=[])
                        new.append(n)
                    si.on_wait = keep
                new.append(inst)
            blk.instructions = new
    return ctr


def _prep_shared(W1, b1, W2, b2, W3, b3):
    L = W1.shape[0]
    # cA row order: [v(0:60) | pad4 | tH(64:94) | pad2 | Hr(96:126)]
    W1A = np.zeros((L, 126, ZL), np.float16)
    W1A[:, 0:VL] = W1[:, :, 30:90].transpose(0, 2, 1)        # v cols
    W1A[:, 64:94] = W1[:, :, 120:150].transpose(0, 2, 1)     # tH cols
    W1A[:, 96:126] = W1[:, :, 0:30].transpose(0, 2, 1)       # Hr cols
    W1B = np.ascontiguousarray(
        W1[:, :, 90:120].transpose(0, 2, 1)).astype(np.float16)  # t cols [L,30,240]

    # rep4 W2: out partition p=(j4*30+k) gets W2 row p%30
    W2T = W2.transpose(0, 2, 1)                              # [L, 240, 30]
    W2R = np.tile(W2T, (1, 1, 4)).astype(np.float16)
    W2RA = np.ascontiguousarray(W2R[:, 0:127])
    W2RB = np.ascontiguousarray(W2R[:, 127:240])
    W3T = W3.transpose(0, 2, 1).astype(np.float16)
    W3A = np.ascontiguousarray(W3T[:, 0:127])
    W3B = np.ascontiguousarray(W3T[:, 127:240])

    BIAS = np.zeros((L, 128, 4), np.float32)
    BIAS[:, 0:127, 0] = b1[:, 0:127]
    BIAS[:, 0:113, 1] = b1[:, 127:240]
    BIAS[:, 0:VL, 2] = b3
    BIAS[:, 0:120, 3] = np.tile(b2, (1, 4))

    # selector: thp[j] = sum_k t[k]*HH[k,j];  p = j4*30+k, group g = j//4
    SEL = np.zeros((120, 8, K), np.float16)
    for j in range(K):
        g, j4 = j // 4, j % 4
        SEL[j4 * K : (j4 + 1) * K, g, j] = 1.0
    return W1A, W1B, W2RA, W2RB, W3A, W3B, BIAS, SEL.reshape(120, 8 * K)


def kernel(Hr, HH, W1, b1, W2, b2, W3, b3, kappa):
    Hr = np.asarray(Hr, np.float32)
    HH = np.asarray(HH, np.float32)
    W1 = np.asarray(W1, np.float32)
    b1 = np.asarray(b1, np.float32)
    W2 = np.asarray(W2, np.float32)
    b2 = np.asarray(b2, np.float32)
    W3 = np.asarray(W3, np.float32)
    b3 = np.asarray(b3, np.float32)
    kappa = np.asarray(kappa, np.float32)

    W1A, W1B, W2RA, W2RB, W3A, W3B, BIAS, SELh = _prep_shared(W1, b1, W2, b2, W3, b3)
    inv_kap = (1.0 / np.abs(kappa)).astype(np.float32)

    in_maps = []
    for ci in range(NCORES):
        sl = slice(ci * BC, (ci + 1) * BC)
        HrT = np.ascontiguousarray(Hr[sl].T).astype(np.float16)
        HHp = np.zeros((BC, K, 32), np.float32)
        HHp[:, :, :K] = HH[sl]
        # HHY[p=(j4*30+k), (g, b)] = HH[b, k, g*4+j4]
        HHY = HHp.reshape(BC, K, 8, 4).transpose(3, 1, 2, 0).reshape(120, 8 * BC)
        in_maps.append({
            "HrT": HrT, "HHY": HHY.astype(np.float16),
            "W1A": W1A, "W1B": W1B, "W2RA": W2RA, "W2RB": W2RB,
            "W3A": W3A, "W3B": W3B,
            "BIAS": BIAS, "SEL": SELh,
            "ZERO32": np.zeros((126, BC), np.float16),
            "ZERO16": np.zeros((120, BC), np.float16),
        })

    nc = build_kernel(inv_kap)
    res = run_bass_kernel_spmd(nc, in_maps, list(range(NCORES)))
    global LAST_RESULT
    LAST_RESULT = res
    out = np.concatenate(
        [r["OUT"].transpose(0, 2, 1) for r in res.results], axis=1
    )
    return np.ascontiguousarray(out.astype(np.float32))
